# revision 7
# baseline (speedup 1.0000x reference)
"""Bass/Trainium2 kernel for nn_DecoderRNN: feedback LSTM decoder.

Math per step (PyTorch LSTMCell, gates (i,f,g,o)):
    gates = x @ W_ih.T + b_ih + h @ W_hh.T + b_hh     x = prev softmax output
    c' = sig(f)*c + sig(i)*tanh(g);  h' = sig(o)*tanh(c')
    y  = softmax(h' @ W_out.T + b_out);  x_next = y
Output is time-reversed: out[T-1-t] = y_t.

Sharding: data-parallel over batch across 8 cores (B=512 -> 64/core),
weights replicated, recurrence local per core.

Device-side design (per core, B=64):
- "H-folded" layout: every per-gate [B, 1024] tensor is stored as
  [128, 512] with partition p = j*64 + b (j = h-half).  This fills all
  128 partitions/PE columns even though the per-core batch is only 64,
  and keeps every elementwise op lane-local.
- gates are computed per-gate (chunks of 512 h-features x 2 halves):
  stationary = xT/hT k-tiles [128, 64]; the two h-halves run as
  concurrent column-group matmuls (tile positions (0,0) / (0,64))
  accumulating into one PSUM bank.
- gate order permuted to (i, f, o, g); sigmoid computed as
  0.5 + 0.5*tanh(x/2) so only the exp_and_others ACT table set is used
  (tanh + exp; no table swaps).
- b_ih+b_hh folded into W_ih.T rows (softmax x sums to exactly 1, so
  adding b to every row of W_ih.T adds b*sum(x) = b).  Step 0 has x=0,
  so its bias comes from K=1 ones-matmuls against a bias row instead.
- h' kept in the folded [128, 512] layout; each [128,128] PE transpose
  then yields TWO hT k-tiles at once (free dim = (j, b)), so 4
  transposes instead of 8 produce the full hT stationary.
- logits computed in a split layout [128=(o_half, b), 256] as
  column-group matmul pairs (N=256 each); b_out via a K=1 ones-pair.
  Softmax across the split needs the two per-partition exp-sums of
  batch b (partitions b and b+64) added: two tiny SBUF->SBUF DMAs swap
  the halves, then one DVE add + reciprocal.
- y [128, 256] (split layout): each [128,128] PE transpose yields TWO
  xT k-tiles, so 2 transposes instead of 4.
"""

import numpy as np
import ml_dtypes

B = 64          # batch per core
H = 1024
HF = 512        # folded h-half size
O = 512
G = 4 * H       # 4096
T = 256
KH = H // 128   # 8 h k-tiles
KX = O // 128   # 4 x k-tiles
NCORES = 8

_BF16 = ml_dtypes.bfloat16

_cache = {}

# Number of steps actually emitted (out buffer stays [T, B, O]); test
# harnesses may lower this to build a transfer-identical baseline module.
T_LIVE = None
# When set (int R), wraps the whole step loop in a hardware For_i loop so
# the body executes R times — used to measure per-step time above host
# noise.  Output values are garbage after the first iteration.
TIMING_REPS = None


def _build():
    import concourse.bass as bass
    import concourse.tile as tile
    from concourse import bacc, mybir

    f32 = mybir.dt.float32
    bf16 = mybir.dt.bfloat16
    Tanh = mybir.ActivationFunctionType.Tanh
    Exp = mybir.ActivationFunctionType.Exp

    nc = bacc.Bacc("TRN2", target_bir_lowering=False, debug=False,
                   num_devices=NCORES)

    # ---- DRAM I/O ----
    # wih: [128, KX*4*2*512]  (k-tile, gate, h-half, h-col), bias folded in
    # whh: [128, KH*4*2*512]
    # wout: [128, KH*512]  (k-tile, o-half, o-col-in-half implicitly 2*256)
    wih_d = nc.dram_tensor("wih", [128, KX * G], bf16, kind="ExternalInput")
    whh_d = nc.dram_tensor("whh", [128, KH * G], bf16, kind="ExternalInput")
    wout_d = nc.dram_tensor("wout", [128, KH * O], bf16, kind="ExternalInput")
    biasrow_d = nc.dram_tensor("biasrow", [1, G], bf16, kind="ExternalInput")
    boutrow_d = nc.dram_tensor("boutrow", [1, O], bf16, kind="ExternalInput")
    onesrow_d = nc.dram_tensor("onesrow", [1, B], bf16, kind="ExternalInput")
    identf_d = nc.dram_tensor("identf", [128, 128], f32, kind="ExternalInput")
    identb_d = nc.dram_tensor("identb", [128, 128], bf16,
                              kind="ExternalInput")
    # h0t: [128, 4, 128]  (p, q, (half, b)) — see hT layout below
    h0t_d = nc.dram_tensor("h0t", [128, 4 * 128], bf16, kind="ExternalInput")
    c0_d = nc.dram_tensor("c0", [128, HF], f32, kind="ExternalInput")
    out_d = nc.dram_tensor("out", [T, B, O], f32, kind="ExternalOutput")

    with tile.TileContext(nc) as tc:
        with (
            tc.tile_pool(name="consts", bufs=1) as consts,
            tc.tile_pool(name="state_c", bufs=2) as state_c,
            tc.tile_pool(name="state_ht", bufs=2) as state_ht,
            tc.tile_pool(name="state_xt", bufs=2) as state_xt,
            tc.tile_pool(name="work", bufs=2) as work,
            tc.tile_pool(name="ys", bufs=3) as ys,
            tc.tile_pool(name="ssums", bufs=3) as ssums,
            tc.tile_pool(name="psum_g", bufs=4, space="PSUM") as psum_g,
            tc.tile_pool(name="psum_l", bufs=1, space="PSUM") as psum_l,
            tc.tile_pool(name="psum_t", bufs=1, space="PSUM") as psum_t,
        ):
            # ---- load constants ----
            wih = consts.tile([128, KX * G], bf16)
            nc.sync.dma_start(out=wih, in_=wih_d[:, :])
            whh = consts.tile([128, KH * G], bf16)
            nc.sync.dma_start(out=whh, in_=whh_d[:, :])
            wout = consts.tile([128, KH * O], bf16)
            nc.sync.dma_start(out=wout, in_=wout_d[:, :])
            biasrow = consts.tile([1, G], bf16)
            nc.sync.dma_start(out=biasrow, in_=biasrow_d[:, :])
            boutrow = consts.tile([1, O], bf16)
            nc.sync.dma_start(out=boutrow, in_=boutrow_d[:, :])
            onesrow = consts.tile([1, B], bf16)
            nc.sync.dma_start(out=onesrow, in_=onesrow_d[:, :])
            identf = consts.tile([128, 128], f32)
            nc.sync.dma_start(out=identf, in_=identf_d[:, :])
            identb = consts.tile([128, 128], bf16)
            nc.sync.dma_start(out=identb, in_=identb_d[:, :])

            c_prev = state_c.tile([128, HF], f32, tag="c")
            nc.sync.dma_start(out=c_prev, in_=c0_d[:, :])
            # hT layout: [128, 4, 128]; true k-tile k -> slice
            #   [:, k % 4, (k // 4) * 64 : (k // 4) * 64 + 64]
            hT_prev = state_ht.tile([128, 4, 128], bf16, tag="ht")
            nc.sync.dma_start(out=hT_prev, in_=h0t_d[:, :])
            xT_prev = None

            def hts(ht, k):
                q, jh = k % 4, k // 4
                return ht[:, q, jh * 64:jh * 64 + 64]

            def xts(xt, k):
                tq, jh = k % 2, k // 2
                return xt[:, tq, jh * 64:jh * 64 + 64]

            def wslice(w, k, g, j):
                # weight block for k-tile k, gate g, h-half j: [128, 512]
                base = ((k * 4 + g) * 2 + j) * HF
                return w[:, base:base + HF]

            t_live = T if T_LIVE is None else T_LIVE
            from contextlib import nullcontext
            loop_ctx = (tc.For_i(0, int(TIMING_REPS), 1)
                        if TIMING_REPS else nullcontext())
            with loop_ctx:
                pend_y = None
                for t in range(t_live):
                    # ---------------- gates: h-part (all 4 chunks) ------
                    tg = work.tile([128, 4, HF], bf16, tag="tg")
                    sg = work.tile([128, 3, HF], bf16, tag="sg")
                    pgs = {}
                    for g in (0, 3, 1, 2):  # emission order: i, g, f, o
                        pg = psum_g.tile([128, HF], f32, tag="pg")
                        pgs[g] = pg
                        for k in range(KH):
                            nc.tensor.matmul(pg[0:B, :],
                                             hts(hT_prev, k),
                                             wslice(whh, k, g, 0),
                                             start=(k == 0), stop=False,
                                             skip_group_check=True)
                            nc.tensor.matmul(pg[B:128, :],
                                             hts(hT_prev, k),
                                             wslice(whh, k, g, 1),
                                             start=(k == 0), stop=False,
                                             skip_group_check=True)

                    # ---- previous step's y -> xT transposes ----
                    # y is [128, 256] split layout; each [128,128]
                    # transpose yields x k-tiles (tq) and (tq+2).
                    if pend_y is not None:
                        y_prev = pend_y
                        ptry = psum_t.tile([128, 2, 128], f32, tag="ptry")
                        xT_new = state_xt.tile([128, 2, 128], bf16,
                                               tag="xt")
                        for tq in range(2):
                            nc.tensor.transpose(
                                ptry[:, tq, :],
                                y_prev[:, tq * 128:(tq + 1) * 128], identf)
                            nc.vector.tensor_copy(
                                out=xT_new[:, tq, :],
                                in_=ptry[:, tq, :])
                        xT_prev = xT_new
                        pend_y = None

                    # ---------------- gates: x-part + activations -------
                    for g in (0, 3, 1, 2):
                        pg = pgs[g]
                        if t == 0:  # bias via K=1 ones-matmul
                            b0 = (g * 2) * HF
                            nc.tensor.matmul(pg[0:B, :], onesrow,
                                             biasrow[:, b0:b0 + HF],
                                             start=False, stop=True,
                                             skip_group_check=True)
                            nc.tensor.matmul(pg[B:128, :], onesrow,
                                             biasrow[:, b0 + HF:b0 + 2 * HF],
                                             start=False, stop=True,
                                             skip_group_check=True)
                        else:
                            for k in range(KX):
                                last = k == KX - 1
                                nc.tensor.matmul(
                                    pg[0:B, :],
                                    xts(xT_prev, k),
                                    wslice(wih, k, g, 0),
                                    start=False, stop=last,
                                    skip_group_check=True)
                                nc.tensor.matmul(
                                    pg[B:128, :],
                                    xts(xT_prev, k),
                                    wslice(wih, k, g, 1),
                                    start=False, stop=last,
                                    skip_group_check=True)
                        # tanh for this gate (x/2 for i,f,o)
                        nc.scalar.activation(
                            out=tg[:, g, :], in_=pg, func=Tanh,
                            scale=0.5 if g < 3 else 1.0)
                        if g < 3:  # sigmoid:  s = 0.5*tanh + 0.5
                            nc.vector.tensor_scalar(
                                out=sg[:, g, :], in0=tg[:, g, :],
                                scalar1=0.5, scalar2=0.5,
                                op0=mybir.AluOpType.mult,
                                op1=mybir.AluOpType.add)

                    # ------- c / h update, split in column halves -------
                    # (halved ops pipeline: stage n of half 1 overlaps
                    #  stage n+1 of half 0, shortening the serial spine)
                    u2 = work.tile([128, HF], bf16, tag="u2")
                    nc.gpsimd.tensor_mul(out=u2, in0=sg[:, 0, :],
                                         in1=tg[:, 3, :])
                    u1 = work.tile([128, HF], f32, tag="u1")
                    c_new = state_c.tile([128, HF], f32, tag="c")
                    th = work.tile([128, HF], bf16, tag="th")
                    hn = work.tile([128, HF], bf16, tag="hn")  # folded h'
                    # transpose folded h' -> hT; each [128,128] transpose
                    # yields k-tiles (q) and (q+4); logits pair per k.
                    # Emission interleaved at half granularity so the hT
                    # copies land on the DVE queue right behind their
                    # half's hn (not after the whole c/h update).
                    ptrh = psum_t.tile([128, 4, 128], bf16, tag="ptrh")
                    hT_new = state_ht.tile([128, 4, 128], bf16, tag="ht")
                    pl = psum_l.tile([128, 256], f32, tag="pl")
                    nc.tensor.matmul(pl[0:B, :], onesrow, boutrow[:, 0:256],
                                     start=True, stop=False,
                                     skip_group_check=True)
                    nc.tensor.matmul(pl[B:128, :], onesrow,
                                     boutrow[:, 256:O],
                                     start=True, stop=False,
                                     skip_group_check=True)
                    HQ = HF // 2
                    for half in range(2):
                        cs = slice(half * HQ, (half + 1) * HQ)
                        nc.vector.tensor_mul(out=u1[:, cs],
                                             in0=sg[:, 1, cs],
                                             in1=c_prev[:, cs])
                        nc.vector.tensor_add(out=c_new[:, cs],
                                             in0=u1[:, cs], in1=u2[:, cs])
                        nc.scalar.activation(out=th[:, cs],
                                             in_=c_new[:, cs], func=Tanh)
                        nc.vector.tensor_mul(out=hn[:, cs],
                                             in0=sg[:, 2, cs],
                                             in1=th[:, cs])
                        for q in (2 * half, 2 * half + 1):
                            nc.tensor.transpose(ptrh[:, q, :],
                                                hn[:, q * 128:(q + 1) * 128],
                                                identb)
                            nc.vector.tensor_copy(
                                out=hT_new[:, q, :],
                                in_=ptrh[:, q, :])
                            for k in (q, q + 4):
                                last = k == 7
                                nc.tensor.matmul(pl[0:B, :], hts(hT_new, k),
                                                 wout[:, k * O:k * O + 256],
                                                 start=False, stop=last,
                                                 skip_group_check=True)
                                nc.tensor.matmul(pl[B:128, :],
                                                 hts(hT_new, k),
                                                 wout[:, k * O + 256:
                                                      (k + 1) * O],
                                                 start=False, stop=last,
                                                 skip_group_check=True)

                    # ---------------- softmax (split layout) ------------
                    # partition p = (o_half, b); row sums of both halves
                    # of batch b live at partitions b and b+64: swap via
                    # two tiny SBUF DMAs, add, reciprocal.
                    eu = work.tile([128, 256], f32, tag="eu")
                    ssum = ssums.tile([128, 1], f32, tag="ssum")
                    nc.scalar.activation(out=eu, in_=pl, func=Exp,
                                         accum_out=ssum)
                    ssw = ssums.tile([128, 1], f32, tag="ssw")
                    nc.sync.dma_start(out=ssw[0:B, :], in_=ssum[B:128, :])
                    nc.sync.dma_start(out=ssw[B:128, :], in_=ssum[0:B, :])
                    stot = ssums.tile([128, 1], f32, tag="stot")
                    nc.vector.tensor_add(out=stot, in0=ssum, in1=ssw)
                    sinv = ssums.tile([128, 1], f32, tag="sinv")
                    nc.vector.reciprocal(out=sinv, in_=stot)
                    y = ys.tile([128, 256], f32, tag="y")
                    nc.scalar.mul(out=y[:, 0:128], in_=eu[:, 0:128],
                                  mul=sinv)
                    nc.scalar.mul(out=y[:, 128:256], in_=eu[:, 128:256],
                                  mul=sinv)
                    tt = (T - 1 - t) % T
                    nc.sync.dma_start(out=out_d[tt, :, 0:256],
                                      in_=y[0:B, :])
                    nc.sync.dma_start(out=out_d[tt, :, 256:O],
                                      in_=y[B:128, :])
                    if t < t_live - 1 or TIMING_REPS:
                        pend_y = y

                    c_prev = c_new
                    hT_prev = hT_new

    nc.compile()
    return nc


def _host_prep(h0, c0, W_ih, W_hh, b_ih, b_hh, W_out, b_out):
    """Build per-core input maps (host-side layout transforms)."""
    f32 = np.float32
    h0 = np.asarray(h0, f32).reshape(NCORES * B, H)
    c0 = np.asarray(c0, f32).reshape(NCORES * B, H)
    W_ih = np.asarray(W_ih, f32)
    W_hh = np.asarray(W_hh, f32)
    W_out = np.asarray(W_out, f32)
    b_tot = np.asarray(b_ih, f32) + np.asarray(b_hh, f32)
    b_out = np.asarray(b_out, f32)

    # permute gate order (i, f, g, o) -> (i, f, o, g)
    perm = np.r_[0:H, H:2 * H, 3 * H:4 * H, 2 * H:3 * H]
    Wih_p = W_ih[perm]          # [G, O]
    Whh_p = W_hh[perm]          # [G, H]
    b_p = b_tot[perm]           # [G]

    # weight layout: [p, k, gate, h-half, h-col] flattened to [128, K*G]
    WihT_aug = Wih_p.T + b_p[None, :]           # [O, G]
    wih_host = np.ascontiguousarray(
        WihT_aug.reshape(KX, 128, 4, 2, HF).transpose(1, 0, 2, 3, 4)
    ).reshape(128, KX * G).astype(_BF16)
    whh_host = np.ascontiguousarray(
        Whh_p.T.reshape(KH, 128, 4, 2, HF).transpose(1, 0, 2, 3, 4)
    ).reshape(128, KH * G).astype(_BF16)
    wout_host = np.ascontiguousarray(
        W_out.T.reshape(KH, 128, O).transpose(1, 0, 2)
    ).reshape(128, KH * O).astype(_BF16)
    biasrow = b_p[None, :].astype(_BF16)        # [1, (gate, half, col)]
    boutrow = b_out[None, :].astype(_BF16)
    onesrow = np.ones((1, B), _BF16)
    identf = np.eye(128, dtype=f32)
    identb = np.eye(128).astype(_BF16)

    in_maps = []
    for i in range(NCORES):
        sl = slice(i * B, (i + 1) * B)
        h0s = h0[sl]                                # [B, H]
        # hT layout [128, 4, 128]: h0t[p, q, half*64 + b]
        #   = h0[b, half*512 + q*128 + p]
        h0t = np.ascontiguousarray(
            h0s.reshape(B, 2, 4, 128).transpose(3, 2, 1, 0)
        ).reshape(128, 4 * 128)
        c0f = np.ascontiguousarray(
            c0[sl].reshape(B, 2, HF).transpose(1, 0, 2)).reshape(128, HF)
        in_maps.append({
            "wih": wih_host, "whh": whh_host, "wout": wout_host,
            "biasrow": biasrow, "boutrow": boutrow, "onesrow": onesrow,
            "identf": identf, "identb": identb,
            "h0t": h0t.astype(_BF16),
            "c0": c0f,
        })
    return in_maps


def kernel(h0, c0, W_ih, W_hh, b_ih, b_hh, W_out, b_out, out_len):
    from concourse.bass_utils import run_bass_kernel_spmd

    assert int(out_len) == T
    if "nc" not in _cache:
        _cache["nc"] = _build()
    nc = _cache["nc"]
    in_maps = _host_prep(h0, c0, W_ih, W_hh, b_ih, b_hh, W_out, b_out)
    res = run_bass_kernel_spmd(nc, in_maps, core_ids=list(range(NCORES)))
    full = np.empty((T, NCORES * B, O), np.float32)
    for i in range(NCORES):
        full[:, i * B:(i + 1) * B, :] = res.results[i]["out"]
    return full
